# revision 22
# baseline (speedup 1.0000x reference)
"""HSIC loss kernel for Trainium2 (Bass/Tile), 8 NeuronCores SPMD.

Math
----
reference computes, for each pair (i, j) of the 4 experts (each [B, d] =
[4096, 256]):

    hsic_ij = trace(center(X_i X_i^T) @ center(X_j X_j^T)) / (B-1)^2

and returns 0.1 * mean over the 6 pairs.  With H = I - 11^T/B idempotent,

    trace(H K H @ H L H) = || Xc^T Yc ||_F^2,   Xc = X - colmean(X)

so each pair reduces to a [d, d] = [256, 256] cross-covariance:

    C = X^T Y - (1/B) sx sy^T,   sx = X^T 1, sy = Y^T 1
    hsic_ij = ||C||_F^2 / (B-1)^2

Sharding: one pair per core (6 of the 8 cores do unique work; cores 6, 7
duplicate cores 4, 5 so the SPMD program is uniform — their outputs are
ignored).  Each core reads its two experts, computes a single already-scaled
partial scalar, and the host just sums 6 floats.  No collectives.

v4 design (HW-measured: warm MM issue ~109 ns regardless of mode, so
DoubleRow = 2 contraction chunks per instruction = 2x TensorE):
  - host casts inputs to fp8 E4M3 (DoubleRow requires e4/e5; loss rel-err
    1.6e-3 measured, tolerance 2e-2) -> HBM read 2 MB/core.
  - host packs [128, 16, 2, 2, 256]: partition p, DR-chunk t, plane q,
    expert e; row 256t + 128q + p.  2KB contiguous per partition per
    2-chunk DMA; 8 DMAs alternate the SP/Act HWDGE rings.
  - C via 32 DoubleRow matmuls (16 chunks x 2 m-halves), PSUM-accumulated.
  - column sums via 16 DoubleRow IDENTITY matmuls into a [128, 512] PSUM
    bank (f32-exact, frees the DVE entirely; identity ships as a tiny
    second input).
  - ~10 junk warm-up matmuls while the first DMA lands flip the PE HAM
    clock-gate to 2.4 GHz before the real stream starts.
"""

import sys

sys.path.insert(0, "/opt/trn_rl_repo")

import ml_dtypes
import numpy as np

B = 4096
D = 256
P = 128
T_CHUNKS = 16  # DoubleRow chunks of 256 rows
DMA_SPLIT = 8
WEIGHT = 0.1
N_PAIRS = 6
SCALE = WEIGHT / N_PAIRS / float(B - 1) ** 2
N_WARMUP = 6

PAIRS = [(0, 1), (0, 2), (0, 3), (1, 2), (1, 3), (2, 3)]
CORE_PAIRS = PAIRS + [PAIRS[4], PAIRS[5]]

_cache = {}


def _patch_drain_split():
    """walrus rejects instructions with >1 sync wait on TRN2 (the Events
    header fits one wait).  Tile's kernel-tail drain aggregates a wait per
    logical proc (12 here).  Split them onto single-wait sync-engine nops
    emitted just before the drain."""
    import concourse.tile as tile
    from concourse.tile import ScopedClock
    from concourse.tile_scheduler import N_PROCS
    from concourse.vector_clock import VectorClock

    if getattr(tile.TileContext, "_drain_split_patched", False):
        return

    def _drain_and_barrier(self, tick_clock, wait_clock):
        gc = tick_clock.global_clock
        for p in range(N_PROCS):
            if gc[p] <= 0:
                continue
            single = VectorClock([gc[q] if q == p else 0 for q in range(N_PROCS)])
            nop = self.nc.sync.nop()
            wait_clock.add_sem_waits(nop.ins, ScopedClock({None: single}))
        self.nc.sync.drain()
        self.nc.all_engine_barrier()
        assert self.sems is not None
        popped = self.nc._tile_sem_poison_stack.pop()
        assert popped is self._sem_poison
        self.nc.clear_and_free_semaphores(list(self.sems.allocated().values()))
        # no second all_engine_barrier after the clear: the measured tail is
        # dominated by the NEXT execution's eagerly-run engine preambles
        # (per-sem clears + start barrier), which also make the second
        # barrier redundant for cross-execution semaphore hygiene.

    tile.TileContext._drain_and_barrier = _drain_and_barrier
    tile.TileContext._drain_split_patched = True


def _build():
    """Build and return (nc, in_name, ident_name, out_name)."""
    from contextlib import ExitStack

    import concourse.bass as bass
    import concourse.tile as tile
    from concourse import mybir

    _patch_drain_split()

    nc = bass.Bass("TRN2")
    inp = nc.dram_tensor(
        [P, T_CHUNKS, 2, 2, D], mybir.dt.float8e4, kind="ExternalInput"
    )
    ident = nc.dram_tensor([P, 2, P], mybir.dt.float8e4, kind="ExternalInput")
    out = nc.dram_tensor([1, 1], mybir.dt.float32, kind="ExternalOutput")

    DR = mybir.MatmulPerfMode.DoubleRow
    TC = T_CHUNKS // DMA_SPLIT  # DR-chunks per DMA tile

    with ExitStack() as ctx:
        tc = ctx.enter_context(tile.TileContext(nc))
        pool = ctx.enter_context(tc.tile_pool(name="pool", bufs=DMA_SPLIT))
        fin = ctx.enter_context(tc.tile_pool(name="fin", bufs=1))
        psum = ctx.enter_context(tc.tile_pool(name="psum", bufs=1, space="PSUM"))

        ones_bf = fin.tile([P, 1], mybir.dt.bfloat16)
        nc.vector.memset(ones_bf[:], 1.0)
        ones_f32 = fin.tile([P, 1], mybir.dt.float32)
        nc.vector.memset(ones_f32[:], 1.0)
        junk = fin.tile([P, 2, D], mybir.dt.float8e4)
        nc.vector.memset(junk[:], 0.25)

        # identity (both DR planes) for the column-sum matmuls, via sync so
        # tile-0 consumers share one DMA-queue semaphore with it
        id_t = fin.tile([P, 2, P], mybir.dt.float8e4)
        nc.sync.dma_start(id_t[:], ident[:])

        tiles = []
        for j in range(DMA_SPLIT):
            t = pool.tile([P, TC, 2, 2, D], mybir.dt.float8e4, tag="in")
            eng = nc.sync if j % 2 == 0 else nc.scalar
            eng.dma_start(t[:], inp[:, j * TC : (j + 1) * TC])
            tiles.append(t)

        g0 = psum.tile([P, D], mybir.dt.float32)
        g1 = psum.tile([P, D], mybir.dt.float32)
        # column-sum accumulators split in two so the first half's
        # partition-reduce overlaps the second half of the stream
        s_bigA = psum.tile([P, 2 * D], mybir.dt.float32)
        s_bigB = psum.tile([P, 2 * D], mybir.dt.float32)
        wup = psum.tile([P, 2 * D], mybir.dt.float32)

        # HAM warm-up: junk matmuls while the first DMAs land
        for w in range(N_WARMUP):
            nc.tensor.matmul(
                wup[:], junk[:, 0, 0:P], junk[:].rearrange("p a b -> p (a b)"),
                start=True, stop=True,
            )

        # sums pipeline: TensorE identity-matmuls cover chunks 0-13 in two
        # accumulators (closing at chunks 6 and 13, partition-reduced under
        # the stream); the idle DVE pre-sums chunks 12-15's tiles so NO sums
        # work trails the last C-matmul.  (s_bigB actually only needs 7-13,
        # and the DVE covers 12-15, so chunks 12-13 are counted once each:
        # s_bigB takes 7-11, DVE takes 12-15.)
        s = psum.tile([1, 2 * D], mybir.dt.float32)
        sbA = fin.tile([P, 2 * D], mybir.dt.bfloat16)
        sbB = fin.tile([P, 2 * D], mybir.dt.bfloat16)
        pair1 = fin.tile([P, 4 * D], mybir.dt.bfloat16)
        pair2 = fin.tile([P, 4 * D], mybir.dt.bfloat16)
        pf = fin.tile([P, 2 * D], mybir.dt.bfloat16)
        A_STOP = 6
        B_STOP = 11
        for t_i in range(T_CHUNKS):
            j, tt = divmod(t_i, TC)
            t = tiles[j]
            first = t_i == 0
            # lhsT [K=128, plane=2, M=128] (plane stride 1024, m stride 1)
            lhs0 = t[:, tt, :, 0, 0:P]
            lhs1 = t[:, tt, :, 0, P:D]
            # rhs [K=128, plane=2, N=256]
            rhs = t[:, tt, :, 1, :]
            nc.tensor.matmul(g0[:], lhs0, rhs, start=first, stop=False,
                             perf_mode=DR)
            nc.tensor.matmul(g1[:], lhs1, rhs, start=first, stop=False,
                             perf_mode=DR)
            # column sums: identity DR-matmul, rhs = both planes' [X|Y]
            if t_i <= B_STOP:
                s_big = s_bigA if t_i <= A_STOP else s_bigB
                nc.tensor.matmul(
                    s_big[:], id_t[:], t[:, tt, :, :, :],
                    start=(t_i in (0, A_STOP + 1)),
                    stop=(t_i in (A_STOP, B_STOP)),
                    perf_mode=DR,
                )
            if t_i == A_STOP:
                nc.vector.tensor_copy(sbA[:], s_bigA[:])
            if t_i == A_STOP + 3:
                nc.tensor.matmul(s[:], ones_bf[:], sbA[:], start=True,
                                 stop=False)
            if t_i == B_STOP:
                # DVE: sum tiles 6,7 (chunks 12-15), both planes
                t6, t7 = tiles[6], tiles[7]
                nc.vector.tensor_add(
                    pair1[:], t6[:, 0, :, :, :], t6[:, 1, :, :, :]
                )
                nc.vector.tensor_add(
                    pair2[:], t7[:, 0, :, :, :], t7[:, 1, :, :, :]
                )
                nc.vector.tensor_add(pair1[:], pair1[:], pair2[:])
                pv = pair1[:].rearrange("p (q c) -> p q c", q=2)
                nc.vector.tensor_add(pf[:], pv[:, 0, :], pv[:, 1, :])
                nc.vector.tensor_copy(sbB[:], s_bigB[:])
            if t_i == B_STOP + 2:
                nc.tensor.matmul(s[:], ones_bf[:], pf[:], start=False,
                                 stop=False)

        # fold in the 7-11 accumulator; s = [sx^T | sy^T] row
        nc.tensor.matmul(s[:], ones_bf[:], sbB[:], start=False, stop=True)

        sums = fin.tile([1, 2 * D], mybir.dt.bfloat16)
        nc.vector.tensor_copy(sums[:], s[:])
        syn = fin.tile([1, D], mybir.dt.bfloat16)
        nc.vector.tensor_scalar_mul(syn[:], s[0:1, D : 2 * D], -1.0 / B)
        nc.tensor.matmul(g0[:], sums[:, 0:P], syn[:], start=False, stop=True)
        nc.tensor.matmul(g1[:], sums[:, P:D], syn[:], start=False, stop=True)

        # sum of squares: ScalarE Square with per-partition accumulation
        sq_scratch0 = fin.tile([P, D], mybir.dt.float32)
        sq_scratch1 = fin.tile([P, D], mybir.dt.float32)
        sq0 = fin.tile([P, 1], mybir.dt.float32)
        sq1 = fin.tile([P, 1], mybir.dt.float32)
        nc.scalar.activation(
            sq_scratch0[:], g0[:], mybir.ActivationFunctionType.Square,
            accum_out=sq0[:],
        )
        nc.scalar.activation(
            sq_scratch1[:], g1[:], mybir.ActivationFunctionType.Square,
            accum_out=sq1[:],
        )
        sqt = fin.tile([P, 1], mybir.dt.float32)
        nc.vector.tensor_add(sqt[:], sq0[:], sq1[:])

        r = psum.tile([1, 1], mybir.dt.float32)
        nc.tensor.matmul(r[:], sqt[:], ones_f32[:], start=True, stop=True)

        res = fin.tile([1, 1], mybir.dt.float32)
        nc.vector.tensor_scalar_mul(res[:], r[:], SCALE)
        nc.gpsimd.dma_start(out[:], res[:])

    return nc, inp.name, ident.name, out.name


def _pack(Xq, Yq):
    """[4096, 256] e4m3 x2 -> [128, 16, 2, 2, 256]: partition p, DR-chunk t,
    plane q, expert e holds row 256t + 128q + p."""
    E = np.stack([Xq, Yq], axis=1)  # [4096, 2, 256]
    E = E.reshape(T_CHUNKS, 2, P, 2, D)  # [t, q, p, e, c]
    return np.ascontiguousarray(E.transpose(2, 0, 1, 3, 4))


def kernel(e0, e1, e2, e3):
    from concourse import bass_utils

    if "built" not in _cache:
        _cache["built"] = _build()
    nc, in_name, id_name, out_name = _cache["built"]

    f8 = ml_dtypes.float8_e4m3
    experts = [
        np.asarray(e, dtype=np.float32).astype(f8) for e in (e0, e1, e2, e3)
    ]
    identity = np.zeros((P, 2, P), dtype=f8)
    for p in range(P):
        identity[p, :, p] = 1.0
    in_maps = [
        {in_name: _pack(experts[a], experts[b]), id_name: identity}
        for (a, b) in CORE_PAIRS
    ]
    res = bass_utils.run_bass_kernel_spmd(nc, in_maps, core_ids=list(range(8)))
    total = np.float32(0.0)
    for c in range(N_PAIRS):
        total += res.results[c][out_name].reshape(())
    return np.asarray(total, dtype=np.float32).reshape(())


if __name__ == "__main__":
    rng = np.random.default_rng(0)
    ins = {f"e{i}": rng.standard_normal((B, D), dtype=np.float32) for i in range(4)}
    print(kernel(**ins))


# revision 23
# speedup vs baseline: 1.0843x; 1.0843x over previous
"""HSIC loss kernel for Trainium2 (Bass/Tile), 8 NeuronCores SPMD.

Math
----
reference computes, for each pair (i, j) of the 4 experts (each [B, d] =
[4096, 256]):

    hsic_ij = trace(center(X_i X_i^T) @ center(X_j X_j^T)) / (B-1)^2

and returns 0.1 * mean over the 6 pairs.  With H = I - 11^T/B idempotent,

    trace(H K H @ H L H) = || Xc^T Yc ||_F^2,   Xc = X - colmean(X)

so each pair reduces to a [d, d] = [256, 256] cross-covariance:

    C = X^T Y - (1/B) sx sy^T,   sx = X^T 1, sy = Y^T 1
    hsic_ij = ||C||_F^2 / (B-1)^2

Sharding: one pair per core (6 of the 8 cores do unique work; cores 6, 7
duplicate cores 4, 5 so the SPMD program is uniform — their outputs are
ignored).  Each core reads its two experts, computes a single already-scaled
partial scalar, and the host just sums 6 floats.  No collectives.

v4 design (HW-measured: warm MM issue ~109 ns regardless of mode, so
DoubleRow = 2 contraction chunks per instruction = 2x TensorE):
  - host casts inputs to fp8 E4M3 (DoubleRow requires e4/e5; loss rel-err
    1.6e-3 measured, tolerance 2e-2) -> HBM read 2 MB/core.
  - host packs [128, 16, 2, 2, 256]: partition p, DR-chunk t, plane q,
    expert e; row 256t + 128q + p.  2KB contiguous per partition per
    2-chunk DMA; 8 DMAs alternate the SP/Act HWDGE rings.
  - C via 32 DoubleRow matmuls (16 chunks x 2 m-halves), PSUM-accumulated.
  - column sums via 16 DoubleRow IDENTITY matmuls into a [128, 512] PSUM
    bank (f32-exact, frees the DVE entirely; identity ships as a tiny
    second input).
  - ~10 junk warm-up matmuls while the first DMA lands flip the PE HAM
    clock-gate to 2.4 GHz before the real stream starts.
"""

import sys

sys.path.insert(0, "/opt/trn_rl_repo")

import ml_dtypes
import numpy as np

B = 4096
D = 256
P = 128
T_CHUNKS = 16  # DoubleRow chunks of 256 rows
DMA_SPLIT = 8
WEIGHT = 0.1
N_PAIRS = 6
SCALE = WEIGHT / N_PAIRS / float(B - 1) ** 2
N_WARMUP = 10

PAIRS = [(0, 1), (0, 2), (0, 3), (1, 2), (1, 3), (2, 3)]
CORE_PAIRS = PAIRS + [PAIRS[4], PAIRS[5]]

_cache = {}


def _patch_drain_split():
    """walrus rejects instructions with >1 sync wait on TRN2 (the Events
    header fits one wait).  Tile's kernel-tail drain aggregates a wait per
    logical proc (12 here).  Split them onto single-wait sync-engine nops
    emitted just before the drain."""
    import concourse.tile as tile
    from concourse.tile import ScopedClock
    from concourse.tile_scheduler import N_PROCS
    from concourse.vector_clock import VectorClock

    if getattr(tile.TileContext, "_drain_split_patched", False):
        return

    def _drain_and_barrier(self, tick_clock, wait_clock):
        gc = tick_clock.global_clock
        for p in range(N_PROCS):
            if gc[p] <= 0:
                continue
            single = VectorClock([gc[q] if q == p else 0 for q in range(N_PROCS)])
            nop = self.nc.sync.nop()
            wait_clock.add_sem_waits(nop.ins, ScopedClock({None: single}))
        self.nc.sync.drain()
        self.nc.all_engine_barrier()
        assert self.sems is not None
        popped = self.nc._tile_sem_poison_stack.pop()
        assert popped is self._sem_poison
        self.nc.clear_and_free_semaphores(list(self.sems.allocated().values()))
        # no second all_engine_barrier after the clear: the measured tail is
        # dominated by the NEXT execution's eagerly-run engine preambles
        # (per-sem clears + start barrier), which also make the second
        # barrier redundant for cross-execution semaphore hygiene.

    tile.TileContext._drain_and_barrier = _drain_and_barrier
    tile.TileContext._drain_split_patched = True


def _build():
    """Build and return (nc, in_name, ident_name, out_name)."""
    from contextlib import ExitStack

    import concourse.bass as bass
    import concourse.tile as tile
    from concourse import mybir

    _patch_drain_split()

    nc = bass.Bass("TRN2")
    inp = nc.dram_tensor(
        [P, T_CHUNKS, 2, 2, D], mybir.dt.float8e4, kind="ExternalInput"
    )
    ident = nc.dram_tensor([P, 2, P], mybir.dt.float8e4, kind="ExternalInput")
    out = nc.dram_tensor([1, 1], mybir.dt.float32, kind="ExternalOutput")

    DR = mybir.MatmulPerfMode.DoubleRow
    TC = T_CHUNKS // DMA_SPLIT  # DR-chunks per DMA tile

    with ExitStack() as ctx:
        tc = ctx.enter_context(tile.TileContext(nc))
        pool = ctx.enter_context(tc.tile_pool(name="pool", bufs=DMA_SPLIT))
        fin = ctx.enter_context(tc.tile_pool(name="fin", bufs=1))
        psum = ctx.enter_context(tc.tile_pool(name="psum", bufs=1, space="PSUM"))

        ones_bf = fin.tile([P, 1], mybir.dt.bfloat16)
        nc.vector.memset(ones_bf[:], 1.0)
        ones_f32 = fin.tile([P, 1], mybir.dt.float32)
        nc.vector.memset(ones_f32[:], 1.0)
        junk = fin.tile([P, 2, D], mybir.dt.float8e4)
        nc.vector.memset(junk[:], 0.25)

        # identity (both DR planes) for the column-sum matmuls, via sync so
        # tile-0 consumers share one DMA-queue semaphore with it
        id_t = fin.tile([P, 2, P], mybir.dt.float8e4)
        nc.sync.dma_start(id_t[:], ident[:])

        tiles = []
        for j in range(DMA_SPLIT):
            t = pool.tile([P, TC, 2, 2, D], mybir.dt.float8e4, tag="in")
            eng = nc.sync if j % 2 == 0 else nc.scalar
            eng.dma_start(t[:], inp[:, j * TC : (j + 1) * TC])
            tiles.append(t)

        g0 = psum.tile([P, D], mybir.dt.float32)
        g1 = psum.tile([P, D], mybir.dt.float32)
        # column-sum accumulators split in two so the first half's
        # partition-reduce overlaps the second half of the stream
        s_bigA = psum.tile([P, 2 * D], mybir.dt.float32)
        s_bigB = psum.tile([P, 2 * D], mybir.dt.float32)
        wup = psum.tile([P, 2 * D], mybir.dt.float32)

        # HAM warm-up: junk matmuls while the first DMAs land
        for w in range(N_WARMUP):
            nc.tensor.matmul(
                wup[:], junk[:, 0, 0:P], junk[:].rearrange("p a b -> p (a b)"),
                start=True, stop=True,
            )

        # sums pipeline: TensorE identity-matmuls cover chunks 0-13 in two
        # accumulators (closing at chunks 6 and 13, partition-reduced under
        # the stream); the idle DVE pre-sums chunks 12-15's tiles so NO sums
        # work trails the last C-matmul.  (s_bigB actually only needs 7-13,
        # and the DVE covers 12-15, so chunks 12-13 are counted once each:
        # s_bigB takes 7-11, DVE takes 12-15.)
        s = psum.tile([1, 2 * D], mybir.dt.float32)
        sbA = fin.tile([P, 2 * D], mybir.dt.bfloat16)
        sbB = fin.tile([P, 2 * D], mybir.dt.bfloat16)
        pair1 = fin.tile([P, 4 * D], mybir.dt.bfloat16)
        pair2 = fin.tile([P, 4 * D], mybir.dt.bfloat16)
        pf = fin.tile([P, 2 * D], mybir.dt.bfloat16)
        A_STOP = 6
        B_STOP = 11
        for t_i in range(T_CHUNKS):
            j, tt = divmod(t_i, TC)
            t = tiles[j]
            first = t_i == 0
            # lhsT [K=128, plane=2, M=128] (plane stride 1024, m stride 1)
            lhs0 = t[:, tt, :, 0, 0:P]
            lhs1 = t[:, tt, :, 0, P:D]
            # rhs [K=128, plane=2, N=256]
            rhs = t[:, tt, :, 1, :]
            nc.tensor.matmul(g0[:], lhs0, rhs, start=first, stop=False,
                             perf_mode=DR)
            nc.tensor.matmul(g1[:], lhs1, rhs, start=first, stop=False,
                             perf_mode=DR)
            # column sums: identity DR-matmul, rhs = both planes' [X|Y]
            if t_i <= B_STOP:
                s_big = s_bigA if t_i <= A_STOP else s_bigB
                nc.tensor.matmul(
                    s_big[:], id_t[:], t[:, tt, :, :, :],
                    start=(t_i in (0, A_STOP + 1)),
                    stop=(t_i in (A_STOP, B_STOP)),
                    perf_mode=DR,
                )
            if t_i == A_STOP:
                nc.vector.tensor_copy(sbA[:], s_bigA[:])
            if t_i == A_STOP + 3:
                nc.tensor.matmul(s[:], ones_bf[:], sbA[:], start=True,
                                 stop=False)
            if t_i == B_STOP:
                # DVE: sum tiles 6,7 (chunks 12-15), both planes
                t6, t7 = tiles[6], tiles[7]
                nc.vector.tensor_add(
                    pair1[:], t6[:, 0, :, :, :], t6[:, 1, :, :, :]
                )
                nc.vector.tensor_add(
                    pair2[:], t7[:, 0, :, :, :], t7[:, 1, :, :, :]
                )
                nc.vector.tensor_add(pair1[:], pair1[:], pair2[:])
                pv = pair1[:].rearrange("p (q c) -> p q c", q=2)
                nc.vector.tensor_add(pf[:], pv[:, 0, :], pv[:, 1, :])
                nc.vector.tensor_copy(sbB[:], s_bigB[:])
            if t_i == B_STOP + 2:
                nc.tensor.matmul(s[:], ones_bf[:], pf[:], start=False,
                                 stop=False)

        # fold in the 7-11 accumulator; s = [sx^T | sy^T] row
        nc.tensor.matmul(s[:], ones_bf[:], sbB[:], start=False, stop=True)

        sums = fin.tile([1, 2 * D], mybir.dt.bfloat16)
        nc.vector.tensor_copy(sums[:], s[:])
        syn = fin.tile([1, D], mybir.dt.bfloat16)
        nc.vector.tensor_scalar_mul(syn[:], s[0:1, D : 2 * D], -1.0 / B)
        nc.tensor.matmul(g0[:], sums[:, 0:P], syn[:], start=False, stop=True)
        nc.tensor.matmul(g1[:], sums[:, P:D], syn[:], start=False, stop=True)

        # sum of squares: ScalarE Square with per-partition accumulation
        sq_scratch0 = fin.tile([P, D], mybir.dt.float32)
        sq_scratch1 = fin.tile([P, D], mybir.dt.float32)
        sq0 = fin.tile([P, 1], mybir.dt.float32)
        sq1 = fin.tile([P, 1], mybir.dt.float32)
        nc.scalar.activation(
            sq_scratch0[:], g0[:], mybir.ActivationFunctionType.Square,
            accum_out=sq0[:],
        )
        nc.scalar.activation(
            sq_scratch1[:], g1[:], mybir.ActivationFunctionType.Square,
            accum_out=sq1[:],
        )
        sqt = fin.tile([P, 1], mybir.dt.float32)
        nc.vector.tensor_add(sqt[:], sq0[:], sq1[:])

        r = psum.tile([1, 1], mybir.dt.float32)
        nc.tensor.matmul(r[:], sqt[:], ones_f32[:], start=True, stop=True)

        res = fin.tile([1, 1], mybir.dt.float32)
        nc.vector.tensor_scalar_mul(res[:], r[:], SCALE)
        nc.gpsimd.dma_start(out[:], res[:])

    return nc, inp.name, ident.name, out.name


def _pack(Xq, Yq):
    """[4096, 256] e4m3 x2 -> [128, 16, 2, 2, 256]: partition p, DR-chunk t,
    plane q, expert e holds row 256t + 128q + p."""
    E = np.stack([Xq, Yq], axis=1)  # [4096, 2, 256]
    E = E.reshape(T_CHUNKS, 2, P, 2, D)  # [t, q, p, e, c]
    return np.ascontiguousarray(E.transpose(2, 0, 1, 3, 4))


def kernel(e0, e1, e2, e3):
    from concourse import bass_utils

    if "built" not in _cache:
        _cache["built"] = _build()
    nc, in_name, id_name, out_name = _cache["built"]

    f8 = ml_dtypes.float8_e4m3
    experts = [
        np.asarray(e, dtype=np.float32).astype(f8) for e in (e0, e1, e2, e3)
    ]
    identity = np.zeros((P, 2, P), dtype=f8)
    for p in range(P):
        identity[p, :, p] = 1.0
    in_maps = [
        {in_name: _pack(experts[a], experts[b]), id_name: identity}
        for (a, b) in CORE_PAIRS
    ]
    res = bass_utils.run_bass_kernel_spmd(nc, in_maps, core_ids=list(range(8)))
    total = np.float32(0.0)
    for c in range(N_PAIRS):
        total += res.results[c][out_name].reshape(())
    return np.asarray(total, dtype=np.float32).reshape(())


if __name__ == "__main__":
    rng = np.random.default_rng(0)
    ins = {f"e{i}": rng.standard_normal((B, D), dtype=np.float32) for i in range(4)}
    print(kernel(**ins))
